# revision 41
# baseline (speedup 1.0000x reference)
"""Trainium2 Bass kernel for the snake-ordered lattice GRU wavefunction model.

v2 strategy (data-parallel over batch, 8 cores x 128 samples):
  - 64 strictly sequential lattice sites. Per site, pre-activations
    pre = st @ [W1sel|W2sel] are computed with the one-hot selection
    reparametrized as base + sum_i g_i * (st @ D_i), where the 0/1 gates
    g_i are folded into the *lhsT* (transposed-hidden) tiles instead of a
    post-GEMM vector chain: gated ring copies = plain_ring * grow, where
    grow[(p, b)] = g_i[b] is a host-precomputed partition-replicated gate
    tile (bf16, exact 0/1). All 5 terms + the bias (K=1 ones-row matmul)
    then accumulate into a SINGLE PSUM bank per site, so the serial DVE
    scalar_tensor_tensor gating chain of v1 disappears, and so do the PE
    filler matmuls that covered it.
  - Activations (tanh/sigmoid) read the pre bank directly from PSUM on
    ScalarE; du = u*(ht - ms) on VectorE; h enters the bf16 ring via the
    PSUM transpose-accumulate trick hT = T(du) + T(ms).
  - x-gated lhsT copies (needed by site t+1 immediately) run on VectorE;
    y-gated copies (needed ~9-15 sites later) run lazily on GpSimd.
  - c7 chunk = [Wmerge | head] with the deferred-head trick: site t's
    logits are produced by site t+1's c7 GEMM (wc7a/wc7b row-alternation).
  - Softmax/sector-mask/log accumulation runs on host (O(B*64*3)).
"""
import os
import sys
import numpy as np

sys.path.insert(0, '/opt/trn_rl_repo')

B, NX, NY, I, H = 1024, 8, 8, 3, 256
N_TARGET, SZ = 48, 0
NCORES = 8
BC = B // NCORES          # 128 samples per core
NSITES = NX * NY          # 64
RING = 16                 # h ring buffer depth (max hy lookback is 15)

_YG_GPSIMD = os.environ.get("BASS_YG_GPSIMD", "0") == "1"
# Measured on HW: warmup/filler MMs cost more stream time than the PE
# clock-gate (HAM) cold penalty they avoid — keep them off.
_WARMUP_MMS = int(os.environ.get("BASS_WARMUP_MMS", "0"))
_ROW0_FILL = int(os.environ.get("BASS_ROW0_FILL", "0"))

_cached = {}


def _snake_sites():
    sites = []
    for ny in range(NY):
        xs = range(NX) if ny % 2 == 0 else range(NX - 1, -1, -1)
        dx = -1 if ny % 2 == 0 else 1
        for nx in xs:
            sites.append((nx, ny, nx + dx))
    return sites


SITES = _snake_sites()


def _site_meta(t):
    nx, ny, nxn = SITES[t]
    x_act = (t % 8 != 0)
    y_act = (t >= 8)
    t_above = 8 * ny - 1 - (t % 8) if y_act else -1
    terms = ([0, 1] if x_act else []) + ([2, 3] if y_act else [])
    return nx, ny, x_act, y_act, t_above, terms


def _build_program_v2():
    import concourse.tile as tile
    from concourse import bacc, mybir

    f32 = mybir.dt.float32
    bf16 = mybir.dt.bfloat16
    Alu = mybir.AluOpType
    Act = mybir.ActivationFunctionType

    nc = bacc.Bacc("TRN2", target_bir_lowering=False, debug=False,
                   num_devices=NCORES)

    f8 = mybir.dt.float8e4
    DR = mybir.MatmulPerfMode.DoubleRow
    from concourse.bass import AP

    wg_d = nc.dram_tensor("wg", [512, 2048], bf16, kind="ExternalInput").ap()
    wbf_d = nc.dram_tensor("wbf", [512, 512], bf16, kind="ExternalInput").ap()
    wbx_d = nc.dram_tensor("wbx", [512, 512], bf16, kind="ExternalInput").ap()
    wby_d = nc.dram_tensor("wby", [512, 512], bf16, kind="ExternalInput").ap()
    wc7a_d = nc.dram_tensor("wc7a", [512, 262], bf16, kind="ExternalInput").ap()
    wc7b_d = nc.dram_tensor("wc7b", [512, 262], bf16, kind="ExternalInput").ap()
    # fp8 W2-base chunks in DoubleRow pair layout:
    # col = 512*pair + 256*sub + n  <->  W2base[128*(2*pair+sub) + p, n]
    wbf8_d = nc.dram_tensor("wbf8", [128, 1024], f8, kind="ExternalInput").ap()
    wbx8_d = nc.dram_tensor("wbx8", [128, 512], f8, kind="ExternalInput").ap()
    wby8_d = nc.dram_tensor("wby8", [128, 512], f8, kind="ExternalInput").ap()
    ones_d = nc.dram_tensor("ones1", [1, 128], bf16, kind="ExternalInput").ap()
    brow_d = nc.dram_tensor("brow", [1, 512], bf16, kind="ExternalInput").ap()
    ident_d = nc.dram_tensor("ident", [128, 128], f32, kind="ExternalInput").ap()
    grow_d = nc.dram_tensor("grow", [128, NSITES * 512], bf16,
                            kind="ExternalInput").ap()
    logits_d = nc.dram_tensor("logits", [128, NSITES * 6], f32,
                              kind="ExternalOutput").ap()
    # site 63's GRU output + head are finished on the host: ship the raw
    # pre-activations and merge state instead of running the serial
    # acts->du->transpose->head drain on-device
    pre63_d = nc.dram_tensor("pre63", [128, 512], f32,
                             kind="ExternalOutput").ap()
    ms63_d = nc.dram_tensor("ms63", [128, 256], f32,
                            kind="ExternalOutput").ap()

    with tile.TileContext(nc) as tc:
        with (
            tc.tile_pool(name="const", bufs=1) as constp,
            tc.tile_pool(name="work", bufs=3) as workp,
            tc.tile_pool(name="psc", bufs=1, space="PSUM") as pscp,
        ):
            # ---- persistent SBUF tiles ----
            wg_sb = [constp.tile([128, 2048], bf16, tag=f"wg{k}", name=f"wg{k}")
                     for k in range(4)]
            wbf_sb = [constp.tile([128, 512], bf16, tag=f"wbf{k}", name=f"wbf{k}")
                      for k in range(4)]
            wbx_sb = [constp.tile([128, 512], bf16, tag=f"wbx{k}", name=f"wbx{k}")
                      for k in range(4)]
            wby_sb = [constp.tile([128, 512], bf16, tag=f"wby{k}", name=f"wby{k}")
                      for k in range(4)]
            wc7a_sb = [constp.tile([128, 262], bf16, tag=f"wa{k}", name=f"wa{k}")
                       for k in range(4)]
            wc7b_sb = [constp.tile([128, 262], bf16, tag=f"wb{k}", name=f"wb{k}")
                       for k in range(4)]
            wbf8_sb = constp.tile([128, 1024], f8, tag="wbf8")
            wbx8_sb = constp.tile([128, 512], f8, tag="wbx8")
            wby8_sb = constp.tile([128, 512], f8, tag="wby8")
            ones_sb = constp.tile([1, 128], bf16, tag="ones")
            brow_sb = constp.tile([1, 512], bf16, tag="brow")
            ident_sb = constp.tile([128, 128], f32, tag="ident")
            grow_sb = constp.tile([128, NSITES * 512], bf16, tag="grow")
            ring_sb = constp.tile([128, RING * 256], bf16, tag="ring")
            ring8_sb = constp.tile([128, RING * 256], f8, tag="ring8")
            xg_sb = constp.tile([128, 2 * 4 * 256], bf16, tag="xg")
            yg_sb = constp.tile([128, 2 * 4 * 256], bf16, tag="yg")
            logit_sb = constp.tile([128, NSITES * 6], f32, tag="lstage")
            zero_sb = constp.tile([128, 128], bf16, tag="zero")

            # ---- persistent PSUM tiles (7 of 8 banks) ----
            pre_ps = [pscp.tile([128, 512], f32, tag=f"pre{i}", name=f"pre{i}")
                      for i in range(2)]
            c7_ps = [pscp.tile([128, 262], f32, tag=f"c7{i}", name=f"c7ps{i}")
                     for i in range(2)]
            tr_ps = [pscp.tile([128, 256], f32, tag=f"tr{i}", name=f"trps{i}")
                     for i in range(2)]
            fill_ps = pscp.tile([128, 512], f32, tag="fill")

            # ---- loads (bf16/fp8 DRAM -> SBUF, no staging) ----
            # Ordered by first use: sites 1-7 need only x-path k0/k1 weights
            # (wbx, wg x-gate cols 0:1024, wc7a) + grow cols for sites 1-7.
            # Row-start sites never contract x-ktiles of wby/wc7b and row-0
            # sites never contract y-ktiles of wbx, so those are not loaded.
            # DMAs alternate between the two TRN2 HWDGE queues (SP and
            # Activation) so the startup weight stream runs on two rings in
            # parallel instead of serializing on one.
            _dq = [nc.sync, nc.scalar]
            _dn = [0]

            def dma2(dst, src):
                _dq[_dn[0] % 2].dma_start(dst, src)
                _dn[0] += 1

            nc.sync.dma_start(ones_sb[:], ones_d)
            nc.sync.dma_start(brow_sb[:], brow_d)
            nc.scalar.dma_start(ident_sb[:], ident_d)
            for k in (0, 1):
                rows = slice(128 * k, 128 * (k + 1))
                dma2(wbx_sb[k][:], wbx_d[rows, :])
                dma2(wg_sb[k][:, 0:1024], wg_d[rows, 0:1024])
                dma2(wc7a_sb[k][:], wc7a_d[rows, :])
                dma2(grow_sb[:, 1024 * k:1024 * (k + 1)],
                     grow_d[:, 1024 * k:1024 * (k + 1)])
            dma2(grow_sb[:, 2048:4096], grow_d[:, 2048:4096])
            dma2(wbx8_sb[:], wbx8_d)
            # site 8 (row start): wby/wc7b/wg y-gate cols, k2/k3; grow site 8
            for k in (2, 3):
                rows = slice(128 * k, 128 * (k + 1))
                dma2(wby_sb[k][:], wby_d[rows, :])
                dma2(wg_sb[k][:, 1024:2048], wg_d[rows, 1024:2048])
                dma2(wc7b_sb[k][:], wc7b_d[rows, :])
            dma2(grow_sb[:, 4096:4608], grow_d[:, 4096:4608])
            dma2(grow_sb[:, 4608:8192], grow_d[:, 4608:8192])
            dma2(wby8_sb[:], wby8_d)
            # site 9+ (interior): wbf all k, wg remaining cols, wc7a k2/k3
            for k in (2, 3):
                rows = slice(128 * k, 128 * (k + 1))
                dma2(wbf_sb[k][:], wbf_d[rows, :])
                dma2(wg_sb[k][:, 0:1024], wg_d[rows, 0:1024])
                dma2(wc7a_sb[k][:], wc7a_d[rows, :])
            for k in (0, 1):
                rows = slice(128 * k, 128 * (k + 1))
                dma2(wbf_sb[k][:], wbf_d[rows, :])
                dma2(wg_sb[k][:, 1024:2048], wg_d[rows, 1024:2048])
            dma2(wbf8_sb[:], wbf8_d)
            # bulk grow tail stays off the scalar queue (acts need it soon)
            nc.sync.dma_start(grow_sb[:, 8192:16384], grow_d[:, 8192:16384])
            nc.sync.dma_start(grow_sb[:, 16384:24576], grow_d[:, 16384:24576])
            nc.sync.dma_start(grow_sb[:, 24576:32768], grow_d[:, 24576:32768])
            zstg = workp.tile([128, 128], f32, tag="zstg", bufs=1)
            nc.vector.memset(zstg[:], 0.0)
            nc.vector.tensor_copy(zero_sb[:], zstg[:])

            # ---- HAM warmup: ~4us of K=1 matmuls during the weight DMAs
            # (row 0 alone never sustains the ~3.4us of continuous PE
            # activity needed to unthrottle the PE clock gate) ----
            for _ in range(_WARMUP_MMS):
                nc.tensor.matmul(fill_ps[:], ones_sb[:], brow_sb[:],
                                 start=True, stop=True)

            def ring_k(site, k):
                base = (site % RING) * 256 + 128 * k
                return ring_sb[:, base:base + 128]

            def ring8_k(site, k):
                base = (site % RING) * 256 + 128 * k
                return ring8_sb[:, base:base + 128]

            def dr_pair(ap2d, n):
                """[128, 2n] 2D slice -> DoubleRow 3D AP [128, 2, n]."""
                return AP(ap2d.tensor, ap2d.offset,
                          ap2d.ap[:-1] + [[n, 2], [1, n]])

            def ring8_pair(site):
                base = (site % RING) * 256
                return dr_pair(ring8_sb[:, base:base + 256], 128)

            def xg_k(site, g, k):
                base = (site % 2) * 1024 + g * 256 + 128 * k
                return xg_sb[:, base:base + 128]

            def yg_k(site, g, k):
                base = (site % 2) * 1024 + g * 256 + 128 * k
                return yg_sb[:, base:base + 128]

            def grow_g(site, g):
                base = site * 512 + g * 128
                return grow_sb[:, base:base + 128]

            def wide2(ap2d):
                """[128, 256] slice -> 3D [128, 2, 128] view."""
                return AP(ap2d.tensor, ap2d.offset,
                          ap2d.ap[:-1] + [[128, 2], [1, 128]])

            def grow_b2(site, g):
                """gate tile broadcast over the ktile dim: [128, 2(bc), 128]"""
                gap = grow_g(site, g)
                return AP(gap.tensor, gap.offset,
                          gap.ap[:-1] + [[0, 2], [1, 128]])

            def yg_w(site, g):
                base = (site % 2) * 1024 + g * 256
                return wide2(yg_sb[:, base:base + 256])

            def ring_w(site):
                base = (site % RING) * 256
                return wide2(ring_sb[:, base:base + 256])

            ms_tiles = {}
            du_tiles = {}
            pre63_stage = [None]

            def emit_T_and_copies(s):
                """Transpose site s's (du, ms) into tr PSUM, stage bf16 ring
                copies on ScalarE, then the deferred logit copy of s-1."""
                for half in (0, 1):
                    sl = slice(128 * half, 128 * (half + 1))
                    nc.tensor.matmul(tr_ps[s % 2][:, sl], du_tiles[s][:, sl],
                                     ident_sb[:], is_transpose=True,
                                     start=True, stop=False)
                    nc.tensor.matmul(tr_ps[s % 2][:, sl], ms_tiles[s][:, sl],
                                     ident_sb[:], is_transpose=True,
                                     start=False, stop=True)
                    nc.scalar.copy(ring_k(s, half), tr_ps[s % 2][:, sl])
                if s >= 1:
                    nc.scalar.copy(logit_sb[:, 6 * (s - 1):6 * s],
                                   c7_ps[s % 2][:, 256:262])
                for half in (0, 1):
                    sl = slice(128 * half, 128 * (half + 1))
                    nc.scalar.copy(ring8_k(s, half), tr_ps[s % 2][:, sl])

            for t in range(NSITES):
                nx, ny, x_act, y_act, t_above, terms = _site_meta(t)
                wb_sb = (wbf_sb if (x_act and y_act) else
                         (wbx_sb if x_act else wby_sb))
                w7_sb = wc7a_sb if (x_act or t == 0) else wc7b_sb
                pre = pre_ps[t % 2]
                c7 = c7_ps[t % 2]

                # collect pre-bank MMs: (lhsT, rhs) in emission order.
                # Base terms: pre1 half in bf16; pre2 half as one fp8
                # DoubleRow MM per ktile-pair (emitted separately below).
                y_mms = []
                x_mms = []
                if y_act:
                    for k in (2, 3):
                        y_mms.append((ring_k(t_above, k - 2),
                                      wb_sb[k][:, 0:256], pre[:, 0:256]))
                        for g in terms:
                            y_mms.append((yg_k(t, g, k - 2),
                                          wg_sb[k][:, 512 * g:512 * (g + 1)],
                                          pre[:]))
                if x_act:
                    for k in (0, 1):
                        x_mms.append((ring_k(t - 1, k), wb_sb[k][:, 0:256],
                                      pre[:, 0:256]))
                        for g in terms:
                            x_mms.append((xg_k(t, g, k),
                                          wg_sb[k][:, 512 * g:512 * (g + 1)],
                                          pre[:]))
                n_pre = len(y_mms) + len(x_mms) + int(y_act) + int(x_act)

                # c7 MMs
                c7y, c7x = [], []
                if y_act:
                    c7y = [(ring_k(t_above, k - 2), w7_sb[k][:]) for k in (2, 3)]
                if x_act:
                    c7x = [(ring_k(t - 1, k), w7_sb[k][:]) for k in (0, 1)]
                if t == 0:
                    c7x = [(zero_sb[:], wc7a_sb[0][:])]
                n_c7 = len(c7y) + len(c7x)

                # ---- row-0 gap fillers: keep the PE clock gate warm
                # through the glue-bound early sites ----
                if 1 <= t <= 8:
                    for _ in range(_ROW0_FILL):
                        nc.tensor.matmul(fill_ps[:], zero_sb[:],
                                         wg_sb[0][:, 0:512],
                                         start=True, stop=True)

                # ---- bias opens the pre accumulation group ----
                nc.tensor.matmul(pre[:], ones_sb[:], brow_sb[:],
                                 start=True, stop=(n_pre == 0))

                # ---- y-phase MMs (first half), then transposes of t-1 ----
                # Row-start sites (y-only): hy = h(t-1), whose ring slot is
                # staged by emit_T_and_copies(t-1) below — so the yg copies
                # for THIS site must be emitted here, after it, not during
                # site t-1's glue.
                late_yg = y_act and not x_act
                emitted = 0
                half1 = 0 if late_yg else (len(y_mms) + 1) // 2
                for lhsT, rhs, out in y_mms[:half1]:
                    emitted += 1
                    nc.tensor.matmul(out, lhsT, rhs, start=False,
                                     stop=(emitted == n_pre))
                if t >= 1:
                    emit_T_and_copies(t - 1)
                if late_yg:
                    for g in terms:
                        nc.vector.tensor_tensor(
                            yg_w(t, g), ring_w(t_above),
                            grow_b2(t, g), Alu.mult)
                for lhsT, rhs, out in y_mms[half1:]:
                    emitted += 1
                    nc.tensor.matmul(out, lhsT, rhs, start=False,
                                     stop=(emitted == n_pre))
                if y_act:
                    # base2 y-pair: fp8 DoubleRow (W2base rows 256:512)
                    emitted += 1
                    rhs8 = (dr_pair(wbf8_sb[:, 512:1024], 256) if x_act
                            else dr_pair(wby8_sb[:, 0:512], 256))
                    nc.tensor.matmul(pre[:, 256:512], ring8_pair(t_above),
                                     rhs8, start=False,
                                     stop=(emitted == n_pre), perf_mode=DR)
                for j, (lhsT, rhs) in enumerate(c7y):
                    nc.tensor.matmul(c7[:], lhsT, rhs, start=(j == 0),
                                     stop=(j == n_c7 - 1))

                # ---- x-gated lhsT copies for this site (DVE) ----
                # half0 first: ktile-0 x-matmuls depend only on those
                if x_act:
                    for k in (0, 1):
                        for g in terms:
                            nc.vector.tensor_tensor(
                                xg_k(t, g, k), ring_k(t - 1, k),
                                grow_g(t, g), Alu.mult)

                # ---- x-phase MMs ----
                for lhsT, rhs, out in x_mms:
                    emitted += 1
                    nc.tensor.matmul(out, lhsT, rhs, start=False,
                                     stop=(emitted == n_pre))
                if x_act:
                    # base2 x-pair: fp8 DoubleRow (W2base rows 0:256)
                    emitted += 1
                    rhs8 = (dr_pair(wbf8_sb[:, 0:512], 256) if y_act
                            else dr_pair(wbx8_sb[:, 0:512], 256))
                    nc.tensor.matmul(pre[:, 256:512], ring8_pair(t - 1),
                                     rhs8, start=False,
                                     stop=(emitted == n_pre), perf_mode=DR)
                for j, (lhsT, rhs) in enumerate(c7x):
                    nc.tensor.matmul(c7[:], lhsT, rhs,
                                     start=(len(c7y) == 0 and j == 0),
                                     stop=(len(c7y) + j == n_c7 - 1))

                # ---- glue: ms copy, activations, du ----
                ms = workp.tile([128, 256], f32, tag="ms")
                ms_tiles[t] = ms
                nc.vector.tensor_copy(ms[:], c7[:, 0:256])
                if t < NSITES - 1:
                    ht = workp.tile([128, 256], f32, tag="ht")
                    u = workp.tile([128, 256], f32, tag="u")
                    du = workp.tile([128, 256], f32, tag="du")
                    du_tiles[t] = du
                    for hf in (0, 1):
                        sl = slice(128 * hf, 128 * (hf + 1))
                        nc.scalar.activation(ht[:, sl], pre[:, sl], Act.Tanh)
                        nc.scalar.activation(
                            u[:, sl],
                            pre[:, 256 + 128 * hf:256 + 128 * (hf + 1)],
                            Act.Sigmoid)
                        nc.vector.tensor_tensor(du[:, sl], ht[:, sl],
                                                ms[:, sl], Alu.subtract)
                        nc.vector.tensor_tensor(du[:, sl], du[:, sl],
                                                u[:, sl], Alu.mult)
                else:
                    # last site: just stage pre for the host (split across
                    # engines so the two copies run concurrently)
                    pre63s = workp.tile([128, 512], f32, tag="p63", bufs=1)
                    nc.vector.tensor_copy(pre63s[:, 0:256], pre[:, 0:256])
                    nc.scalar.copy(pre63s[:, 256:512], pre[:, 256:512])
                    pre63_stage[0] = pre63s

                # ---- lazy y-gated lhsT copies for site t+1 ----
                # (row-start consumers have ta1 == t and are handled above)
                # ---- stream finished logit chunks back to HBM early ----
                if t in (20, 36, 52):
                    lo = 96 * ((t - 20) // 16)
                    nc.sync.dma_start(logits_d[:, lo:lo + 96],
                                      logit_sb[:, lo:lo + 96])

                yg_engine = nc.gpsimd if _YG_GPSIMD else nc.vector
                if t + 1 < NSITES:
                    _, _, _, y1, ta1, terms1 = _site_meta(t + 1)
                    if y1 and ta1 != t:
                        for g in terms1:
                            yg_engine.tensor_tensor(
                                yg_w(t + 1, g), ring_w(ta1),
                                grow_b2(t + 1, g), Alu.mult)

            # ---- tail: logit(62) from c7(63); ship pre/ms of site 63 ----
            nc.scalar.copy(logit_sb[:, 6 * 62:6 * 63],
                           c7_ps[(NSITES - 1) % 2][:, 256:262])
            nc.sync.dma_start(pre63_d, pre63_stage[0][:])
            nc.sync.dma_start(ms63_d, ms_tiles[NSITES - 1][:])
            nc.sync.dma_start(logits_d[:, 288:378], logit_sb[:, 288:378])

    nc.compile()
    return nc


def _host_pre_v2(samples, W1, W2, Wmerge, Wl1, Wl2, b1, b2):
    from ml_dtypes import bfloat16 as bf16np
    from ml_dtypes import float8_e4m3 as f8np
    oh = np.zeros((B, NX, NY, I), np.float32)
    idx = np.indices(samples.shape)
    oh[idx[0], idx[1], idx[2], samples] = 1.0
    SX = np.zeros((NSITES, B, I), np.float32)
    SY = np.zeros((NSITES, B, I), np.float32)
    for t, (nx, ny, nxn) in enumerate(SITES):
        if 0 <= nxn < NX:
            SX[t] = oh[:, nxn, ny]
        if ny > 0:
            SY[t] = oh[:, nx, ny - 1]

    def DD(i, b):
        return np.concatenate([W1[i] - W1[b], W2[i] - W2[b]], axis=1)

    wg = np.concatenate([DD(1, 0), DD(2, 0), DD(4, 3), DD(5, 3)], axis=1)
    wbf = np.concatenate([W1[0] + W1[3], W2[0] + W2[3]], axis=1)
    wbx = np.concatenate([W1[0], W2[0]], axis=1)
    wby = np.concatenate([W1[3], W2[3]], axis=1)
    Wl = np.concatenate([Wl1, Wl2], axis=1)
    z = np.zeros((H, 6), np.float32)
    wc7a = np.concatenate([Wmerge, np.concatenate([Wl, z], 0)], axis=1)
    wc7b = np.concatenate([Wmerge, np.concatenate([z, Wl], 0)], axis=1)
    brow = np.concatenate([b1, b2]).astype(np.float32)[None, :]
    # fp8 DoubleRow pair layouts for the W2-base (pre2) chunks
    wbf8 = (W2[0] + W2[3]).reshape(4, 128, 256).transpose(1, 0, 2)
    wbx8 = W2[0][0:256].reshape(2, 128, 256).transpose(1, 0, 2)
    wby8 = W2[3][256:512].reshape(2, 128, 256).transpose(1, 0, 2)
    c = lambda x: np.ascontiguousarray(x).astype(bf16np)
    c8 = lambda x, n: np.ascontiguousarray(x).astype(f8np).reshape(128, n)
    return (SX, SY, c(wg), c(wbf), c(wbx), c(wby), c(wc7a), c(wc7b), c(brow),
            c8(wbf8, 1024), c8(wbx8, 512), c8(wby8, 512))


def _host_post(samples, logits, bl1, bl2):
    """logits: [B, NSITES, 6].  Returns (0.5*log_a, log_p)."""
    log_a = np.zeros(B, np.float32)
    log_p = np.zeros(B, np.float32)
    bl_up = (N_TARGET + 2 * SZ) // 2
    bl_dn = (N_TARGET - 2 * SZ) // 2
    bl_hole = NX * NY - N_TARGET
    n_up = np.zeros(B, np.float32)
    n_dn = np.zeros(B, np.float32)
    ar = np.arange(B)
    for t, (nx, ny, nxn) in enumerate(SITES):
        l1 = logits[:, t, 0:3] + bl1
        l2 = logits[:, t, 3:6] + bl2
        e = np.exp(l1 - l1.max(axis=1, keepdims=True))
        probs = e / e.sum(axis=1, keepdims=True)
        phase = np.float32(np.pi) * (l2 / (1.0 + np.abs(l2)))
        m_up = (bl_up - n_up > 0).astype(np.float32)
        m_dn = (bl_dn - n_dn > 0).astype(np.float32)
        m_hole = (bl_hole - (t - n_up - n_dn) > 0).astype(np.float32)
        mask = np.stack([m_hole, m_dn, m_up], axis=1)
        amp = probs * mask
        amp = amp / np.maximum(amp.sum(axis=1, keepdims=True), 1e-30)
        s = samples[:, nx, ny]
        log_a += np.log(np.clip(amp[ar, s], 1e-12, None)).astype(np.float32)
        log_p += phase[ar, s].astype(np.float32)
        n_up += (s == 2)
        n_dn += (s == 1)
    return (0.5 * log_a).astype(np.float32), log_p.astype(np.float32)


last_results = None  # exposed for test.py profiling


def _install_neff_saver(dst_dir):
    """Monkeypatch bass2jax's BIR->NEFF compile to retain a NEFF copy for
    neuron-profile (the axon path normally discards it)."""
    import shutil
    from concourse import bass2jax as b2j
    if getattr(b2j, "_neff_saver_installed", False):
        return
    orig = b2j.compile_bir_kernel

    def wrapper(bir_json, tmpdir, neff_name="file.neff", **kw):
        out = orig(bir_json, tmpdir, neff_name=neff_name, **kw)
        try:
            shutil.copy(out, os.path.join(dst_dir, "kernel.neff"))
        except Exception:
            pass
        return out

    b2j.compile_bir_kernel = wrapper
    b2j._neff_saver_installed = True


def kernel(samples, W1, b1, W2, b2, Wmerge, Wl1, bl1, Wl2, bl2):
    global last_results
    from concourse.bass_utils import run_bass_kernel_spmd
    from ml_dtypes import bfloat16 as bf16np

    samples = np.asarray(samples).astype(np.int64)
    f = lambda x: np.asarray(x, dtype=np.float32)
    W1, b1, W2, b2 = f(W1), f(b1), f(W2), f(b2)
    Wmerge, Wl1, bl1, Wl2, bl2 = f(Wmerge), f(Wl1), f(bl1), f(Wl2), f(bl2)

    (SX, SY, wg, wbf, wbx, wby, wc7a, wc7b, brow,
     wbf8, wbx8, wby8) = _host_pre_v2(
        samples, W1, W2, Wmerge, Wl1, Wl2, b1, b2)

    if "nc" not in _cached:
        _cached["nc"] = _build_program_v2()
    nc = _cached["nc"]

    ident = np.eye(128, dtype=np.float32)
    ones1 = np.ones((1, 128), dtype=bf16np)
    core_ids = list(range(NCORES))
    in_maps = []
    for c in core_ids:
        sl = slice(c * BC, (c + 1) * BC)
        # grow[p, t*512 + g*128 + b] = gate_g(t)[sample b], replicated over p
        gates = np.empty((NSITES, 4, BC), np.float32)
        gates[:, 0] = SX[:, sl, 1]
        gates[:, 1] = SX[:, sl, 2]
        gates[:, 2] = SY[:, sl, 1]
        gates[:, 3] = SY[:, sl, 2]
        grow = np.broadcast_to(gates.reshape(1, NSITES * 4 * BC),
                               (128, NSITES * 4 * BC)).astype(bf16np)
        in_maps.append({"wg": wg, "wbf": wbf, "wbx": wbx, "wby": wby,
                        "wc7a": wc7a, "wc7b": wc7b, "brow": brow,
                        "wbf8": wbf8, "wbx8": wbx8, "wby8": wby8,
                        "ones1": ones1, "ident": ident,
                        "grow": np.ascontiguousarray(grow)})

    ntff_dir = os.environ.get("BASS_NTFF_DIR", "")
    if ntff_dir:
        os.makedirs(ntff_dir, exist_ok=True)
        _install_neff_saver(ntff_dir)
        from trn_agent_boot.trn_boot import _ntff_profile_via_ctypes
        hook = _ntff_profile_via_ctypes("/opt/axon/libaxon_pjrt.so")
        with hook(ntff_dir, None):
            res = run_bass_kernel_spmd(nc, in_maps, core_ids)
    else:
        res = run_bass_kernel_spmd(nc, in_maps, core_ids)
    last_results = res

    logits = np.concatenate(
        [res.results[c]["logits"].reshape(BC, NSITES, 6) for c in core_ids],
        axis=0)
    # finish site 63 (GRU output + head) on the host from raw pre/ms
    for c in core_ids:
        p = np.asarray(res.results[c]["pre63"], np.float32)
        m = np.asarray(res.results[c]["ms63"], np.float32)
        ht = np.tanh(p[:, :256])
        u = 1.0 / (1.0 + np.exp(-p[:, 256:]))
        h = u * ht + (1.0 - u) * m
        logits[c * BC:(c + 1) * BC, 63, 0:3] = h @ Wl1
        logits[c * BC:(c + 1) * BC, 63, 3:6] = h @ Wl2
    return _host_post(samples, logits, bl1, bl2)
